# revision 1
# baseline (speedup 1.0000x reference)
"""L-infinity distance "convolution" kernel for Trainium2 (8 NeuronCores).

Computes out[b, co, h, w] = max_acc |weights[co, acc] - patch[b, h, w, acc]| + bias[co]
where patches are 3x3 replicate-padded windows over x (4, 16, 64, 64),
acc = (c, kh, kw) ordered, accl = 16*9 = 144, cout = 64.

Sharding: 8 cores = 4 batches x 2 row-halves. Each core computes a
[2048 positions, 64 cout] shard. No collectives needed.

Device layout: partitions = 128 spatial positions per tile (16 tiles/core).

Compute: a custom SEGMENTED scan-max DVE op (hand-lowered 3-state uop FSM
whose SUB_DIM_DONE step state re-seeds the scan recurrence at each
[P, S, N] page boundary). One instruction covers a whole cout-chunk:
in0 = replicated weights [P, S couts, 144], in1 = patch row page-broadcast
(page-stride 0), out through an AP whose inner dim has step 0
([[1, S], [0, 144]]) so each page's final running max lands directly in
dist[:, co]. This is 1 cycle/element with ~1% instruction overhead,
fp32-exact. Bias adds run on the otherwise-idle gpsimd.

Weight replication across partitions is done ON-CHIP: one small [1, N]
DMA, then K=1 ones-matmul broadcasts on the idle PE with PSUM->SBUF
copies on the idle ScalarE (a partition-broadcast DMA has ~6 us fixed
latency and Tile merges chunk waits into ~19 us; fp32 x 1.0 through the
PE is bitwise exact, HW-verified). The cout chunk ladder [2, 6, 16, 40]
makes the first chunk consumable early so compute starts at ~7 us; from
tile 4 on, a merged full-width W tile (assembled by the idle gpsimd)
lets each tile run as a single 64-cout instruction.

All arithmetic is fp32 and bit-exact vs the float32 reference (the custom
op computes max(a-b, b-a) = |a-b| exactly; maxes are exact).
TimelineSim cost model: 163.1 us per core. Journey: 331 us (TT-sub +
segmented reduce) -> 237 (fused per-cout scan op) -> 202 (+gpsimd split)
-> 176 (segmented scan) -> 166 (PE weight broadcast + ladder + bias on
gpsimd) -> 163.5 (merged-W single-op tiles + ladder/broadcast tuning). DVE busy is ~156 us =
16 tiles x 9216 elems at 1 cyc/elem — the engine floor for this op set
(2x/4x DVE modes need single-source or 16-bit dtypes). All gpsimd
chain-offload shapes (deep/shallow max-min trees, sub-only) measure
20-80 us above their LP bounds from cross-engine scheduling tax and
lose to the all-DVE schedule; gpsimd serves as epilogue engine only.
"""

import numpy as np

B, C, H, W = 4, 16, 64, 64
K = 3
COUT = 64
ACC = C * K * K  # 144
HOUT, WOUT = 64, 64
NPOS = HOUT * WOUT  # 4096
NCORES = 8
HALVES = 2
POS_PER_CORE = NPOS // HALVES  # 2048
P = 128  # partitions
NTILES = POS_PER_CORE // P  # 16
COG = 8  # cout chunk for weight broadcast tiles
NCHUNK = COUT // COG  # 8

# tuning knobs (A/B-tested via TimelineSim)
CFG = {
    "gps_count": 0,  # all-DVE compute; gpsimd dual-tree tiles only break even
    "gps_tree": 0,  # (unused) legacy knob
    "mix_tiles": 0,  # squash tiles that donate their last cout-chunk to gpsimd
    "gps_whole_w": False,  # gps sub as one op reading a whole-weights tile
    "gps_full": False,  # gpsimd end-to-end tiles (dual max/min tree): ~even
    "gps_bias": True,  # run the gps tiles' bias add on gpsimd instead of DVE
    "sq_bias_gps": True,  # squash tiles' bias add on (idle) gpsimd
    "dwork_bufs": 3,
    "work_bufs": 6,
    "outp_bufs": 6,
    "w_cog": 32,  # cout per weight-chunk tile if w_chunks is None
    "w_chunks": [2, 6, 16, 40],  # cout chunk ladder: small first for early start
    "mmn": 288,  # broadcast matmul chunk columns
    "bias_after": 2,  # emit the bias broadcast after this ladder chunk
    "psum_bufs": 4,
    "flush_keep": 0,  # defer gpsimd tiles' DVE mini-reduces when gps_full
    "wfull_from": 4,  # tiles >= this use one 64-cout op on a merged W tile
}

_TRACE = False

_OP_CACHE = None


def _lower_segscan(spec, ver):
    """Hand-lowered 3-state FSM for a SEGMENTED scan: seed -> steady, with a
    SUB_DIM_DONE step state that re-seeds the scan recurrence on the first
    element of each [P, S, N] page (computing op(init, expr) instead of
    op(carry, expr)). The stock lower() has no per-page reset for regular
    scans; this provides one, giving per-page reductions from one
    instruction. HW-verified bit-exact."""
    import concourse.dve_spec as ds
    from concourse.dve_spec import Trigger

    n_lanes, n_stages = ds.N_LANES[ver], ds.N_STAGES[ver]
    ds._validate_body(spec, ver)
    spec2 = ds._hoist_stream_invariant_ops(spec)
    scans = ds._collect(spec2.body, ds.Scan)
    latches = ds._collect(spec2.body, ds.Latch)
    assert not latches and spec2.accum is None
    p = ds._build_placement(spec2, scans, n_stages, n_lanes)
    seed_ov, step_ov0 = ds._scan_overrides(scans, p.node_stage)
    assert not step_ov0  # regular scans only (no PageIdx)
    step_ov = {}
    for sc in scans:
        d = p.node_stage[sc]
        step_ov[d] = ds._Stage(sc.op, ds._scan_init(sc), sc.expr)
    body_lvs = ds._body_scan_leaves(spec2)
    consume = (ds.Src0 in body_lvs, ds.Src1 in body_lvs)
    states = [
        ds._State(
            placement=p,
            overrides=seed_ov,
            trigger=ds.COUNT_ONCE,
            repeat=1,
            next=(1, 0, 0),
            write_out=False,
        ),
        ds._State(
            placement=p,
            consume=consume,
            trigger=(Trigger.SRC_TENSOR_DONE, Trigger.SUB_DIM_DONE, Trigger.NONE),
            next=(0, 2, 0),
        ),
        ds._State(
            placement=p,
            consume=consume,
            overrides=step_ov,
            trigger=(Trigger.SRC_TENSOR_DONE, Trigger.SUB_DIM_DONE, Trigger.COUNT),
            next=(0, 2, 1),
            repeat=1,
        ),
    ]
    out = [ds._assemble(s) for s in states]
    for u in out:
        u.validate(ver)
    return out


def _get_op():
    """Register (once) the segmented |a-b| scan-max custom DVE op."""
    global _OP_CACHE
    if _OP_CACHE is not None:
        return _OP_CACHE
    from concourse.dve_spec import Spec, Src0, Src1, maxx, AluOp, scan
    from concourse.dve_uop import DveOpSpec
    import concourse.dve_ops as dve_ops
    from concourse.dve_ops import DveOp

    def _ref(in0, in1, s0, s1, imm2):
        b = np.maximum.accumulate(np.abs(in0.astype(np.float32) - in1), axis=-1)
        return b.astype(np.float32)

    spec = Spec(body=scan(AluOp.MAX, maxx(Src0 - Src1, Src1 - Src0)), reference=_ref)
    name = "ABSDIFF_MAX_SEGSCAN"
    if name not in dve_ops._SUB_OPCODE_FOR_NAME:
        row = max(dve_ops._SUB_OPCODE_FOR_NAME.values()) + 1
        assert row < 0x20
        dve_ops._SUB_OPCODE_FOR_NAME[name] = row
    row = dve_ops._SUB_OPCODE_FOR_NAME[name]
    shas = {}
    for ver in ("v3", "v4"):
        s = DveOpSpec(
            name=name, opcode=row, uops=_lower_segscan(spec, ver), rd1_en=True
        )
        # Pre-populate the compile cache so DveOp.compile() returns the
        # hand-lowered program instead of re-running the stock lower().
        dve_ops._COMPILE_CACHE[(name, ver)] = s
        shas[ver] = s.sha(ver)
    op = DveOp(name, spec, subdim=True, uops_sha=shas)
    if all(o.name != name for o in dve_ops.OPS):
        dve_ops.OPS.append(op)
        dve_ops.CUSTOM_DVE_SPECS[name] = spec
    _OP_CACHE = op
    return op


def _build_bass():
    import concourse.bass as bass
    import concourse.bacc as bacc
    import concourse.mybir as mybir
    import concourse.tile as tile
    from concourse.alu_op_type import AluOpType

    op = _get_op()

    nc = bacc.Bacc("TRN2", target_bir_lowering=False, debug=False, num_devices=NCORES)
    patches_d = nc.dram_tensor(
        "patches", [POS_PER_CORE, ACC], mybir.dt.float32, kind="ExternalInput"
    )
    w_d = nc.dram_tensor("w", [1, COUT * ACC], mybir.dt.float32, kind="ExternalInput")
    bias_d = nc.dram_tensor("bias", [1, COUT], mybir.dt.float32, kind="ExternalInput")
    out_d = nc.dram_tensor(
        "out", [POS_PER_CORE, COUT], mybir.dt.float32, kind="ExternalOutput"
    )

    # gpsimd tiles interleaved with DVE tiles; odd positions first so the
    # DVE starts on tile 0 immediately.
    kg = CFG["gps_count"]
    order = list(range(1, NTILES, 2)) + list(range(0, NTILES, 2))
    gps_tiles = set(order[:kg])
    tree_levels = CFG["gps_tree"]
    # squash tiles whose last cout-group is donated to gpsimd
    squash_order = [t for t in order if t not in gps_tiles]
    mix_tiles = set(squash_order[: CFG["mix_tiles"]])

    with tile.TileContext(nc) as tc:
        with (
            tc.tile_pool(name="consts", bufs=1) as consts,
            tc.tile_pool(name="work", bufs=CFG["work_bufs"]) as work,
            tc.tile_pool(name="dwork", bufs=CFG["dwork_bufs"]) as dwork,
            tc.tile_pool(name="outp", bufs=CFG["outp_bufs"]) as outp,
            tc.tile_pool(name="psum", bufs=CFG.get("psum_bufs", 4), space="PSUM") as psp,
        ):
            # Weights replicated across all 128 partitions. A partition-
            # broadcast DMA has ~6 us fixed latency and Tile merges the
            # chunk waits, stalling the first consumer ~19 us. Instead:
            # one small [1, N] DMA, then a K=1 ones-matmul broadcast on the
            # (otherwise idle) PE with PSUM->SBUF copies on the (otherwise
            # idle) ScalarE. fp32 x 1.0 through the PE is bitwise exact
            # (HW-verified). First chunk is consumable in ~8 us and the
            # rest pipelines behind compute.
            chunk_sizes = CFG["w_chunks"] or [CFG["w_cog"]] * (COUT // CFG["w_cog"])
            assert sum(chunk_sizes) == COUT
            starts = [sum(chunk_sizes[:i]) for i in range(len(chunk_sizes))]
            cog = chunk_sizes[-1]  # for the mix-tile donation path
            nch = len(chunk_sizes)
            # prefetch the first patch tiles before anything else queues
            pre_pt = {}
            for t0_ in range(CFG.get("pt_prefetch", 0)):
                ptp = work.tile([P, ACC], mybir.dt.float32, tag="pt")
                nc.sync.dma_start(
                    out=ptp[:, :], in_=patches_d[t0_ * P : (t0_ + 1) * P, :]
                )
                pre_pt[t0_] = ptp
            # SWDGE queue for these so the patch-tile loads (HWDGE) don't
            # queue behind them
            weng = nc.sync if CFG.get("wflat_sync", False) else nc.gpsimd
            # under gps_full, borrow the D tag so wflat's 36KB frees after
            # the broadcast instead of living forever in consts
            if CFG["gps_full"] and kg > 0:
                wflat_s = dwork.tile([1, COUT * ACC], mybir.dt.float32, tag="D", bufs=2)
            else:
                wflat_s = consts.tile([1, COUT * ACC], mybir.dt.float32)
            weng.dma_start(out=wflat_s[:, :], in_=w_d[0:1, :])
            bflat_s = consts.tile([1, COUT], mybir.dt.float32)
            weng.dma_start(out=bflat_s[:, :], in_=bias_d[0:1, :])
            ones = consts.tile([1, P], mybir.dt.float32)
            nc.gpsimd.memset(ones[:, :], 1.0)
            bias_rep = consts.tile([P, COUT], mybir.dt.float32)
            zeros9 = None
            if CFG["gps_full"] and kg > 0:
                zeros9 = consts.tile([P, COUT * 9], mybir.dt.float32)
                nc.gpsimd.memset(zeros9[:, :], 0.0)

            MMN = CFG.get("mmn", 512)
            wchunks = []
            for g in range(nch):
                sz = chunk_sizes[g]
                wt = consts.tile([P, sz * ACC], mybir.dt.float32, tag=f"wch{g}")
                cols = sz * ACC
                for j in range(0, cols, MMN):
                    nn = min(MMN, cols - j)
                    ps = psp.tile([P, MMN], mybir.dt.float32, tag="psb")
                    c0 = starts[g] * ACC + j
                    nc.tensor.matmul(
                        ps[:, 0:nn],
                        ones[:, :],
                        wflat_s[:, c0 : c0 + nn],
                        start=True,
                        stop=True,
                    )
                    nc.scalar.copy(out=wt[:, j : j + nn], in_=ps[:, 0:nn])
                wchunks.append(wt)
                if g == CFG.get("bias_after", 0):
                    psb = psp.tile([P, MMN], mybir.dt.float32, tag="psb")
                    nc.tensor.matmul(
                        psb[:, 0:COUT], ones[:, :], bflat_s[:, :], start=True, stop=True
                    )
                    nc.scalar.copy(out=bias_rep[:, :], in_=psb[:, 0:COUT])

            # merged full-width W copy (built by the idle gpsimd off the
            # ladder chunks) so later tiles need only one segscan instruction
            wfull = None
            wfull_from = CFG.get("wfull_from", NTILES)
            if wfull_from < NTILES:
                wfull = consts.tile([P, COUT * ACC], mybir.dt.float32)
                for g in range(nch):
                    s0 = starts[g] * ACC
                    sz = chunk_sizes[g] * ACC
                    nc.gpsimd.tensor_copy(
                        out=wfull[:, s0 : s0 + sz], in_=wchunks[g][:, :]
                    )
            wbig = None

            pending = []  # gpsimd tiles awaiting their DVE reduce

            def flush_pending(keep=0):
                while len(pending) > keep:
                    t0, r3 = pending.pop(0)
                    dist = outp.tile([P, COUT], mybir.dt.float32, tag="dist")
                    if isinstance(r3, tuple) and r3[0] == "r36":
                        # dual 36-wide remainders: dist = max(max-reduce,
                        # -min-reduce)
                        r36 = r3[1]
                        nc.vector.tensor_reduce(
                            out=dist[:, :],
                            in_=r36[:, 0 : COUT * 36].rearrange(
                                "p (c a) -> p c a", a=36
                            ),
                            axis=mybir.AxisListType.X,
                            op=AluOpType.max,
                        )
                        tmpn = outp.tile([P, COUT], mybir.dt.float32, tag="tmpn", bufs=2)
                        nc.vector.tensor_reduce(
                            out=tmpn[:, :],
                            in_=r36[:, COUT * 36 : COUT * 72].rearrange(
                                "p (c a) -> p c a", a=36
                            ),
                            axis=mybir.AxisListType.X,
                            op=AluOpType.min,
                        )
                        nc.vector.tensor_scalar(
                            out=tmpn[:, :],
                            in0=tmpn[:, :],
                            scalar1=-1.0,
                            scalar2=None,
                            op0=AluOpType.mult,
                        )
                        nc.vector.tensor_tensor(
                            out=dist[:, :],
                            in0=dist[:, :],
                            in1=tmpn[:, :],
                            op=AluOpType.max,
                        )
                    else:
                        nc.vector.tensor_reduce(
                            out=dist[:, :],
                            in_=r3,
                            axis=mybir.AxisListType.X,
                            op=AluOpType.max,
                            apply_absolute_value=True,
                        )
                    bias_eng = nc.gpsimd if CFG["gps_bias"] else nc.vector
                    bias_eng.tensor_tensor(
                        out=dist[:, :],
                        in0=dist[:, :],
                        in1=bias_rep[:, :],
                        op=AluOpType.add,
                    )
                    nc.sync.dma_start(
                        out=out_d[t0 * P : (t0 + 1) * P, :], in_=dist[:, :]
                    )

            for t in range(NTILES):
                if t in pre_pt:
                    pt = pre_pt.pop(t)
                else:
                    pt = work.tile([P, ACC], mybir.dt.float32, tag="pt")
                    pt_eng = (
                        nc.gpsimd
                        if (t == 0 and CFG.get("pt0_swdge", False))
                        else nc.sync
                    )
                    pt_eng.dma_start(
                        out=pt[:, :], in_=patches_d[t * P : (t + 1) * P, :]
                    )
                if t in gps_tiles:
                    # chunked subtract: one gpsimd op per cout-group, reading
                    # its weight chunk + the patch broadcast
                    d_t = dwork.tile([P, COUT * ACC], mybir.dt.float32, tag="D", bufs=2)
                    if wbig is not None:
                        pt_b = pt[:, :].unsqueeze(1).broadcast_to([P, COUT, ACC])
                        nc.gpsimd.tensor_tensor(
                            out=d_t[:, :].rearrange("p (c a) -> p c a", a=ACC),
                            in0=wbig[:, :].rearrange("p (c a) -> p c a", a=ACC),
                            in1=pt_b,
                            op=AluOpType.subtract,
                        )
                    else:
                        for g in range(nch):
                            sz = chunk_sizes[g]
                            s0 = starts[g]
                            pt_b = pt[:, :].unsqueeze(1).broadcast_to([P, sz, ACC])
                            d3g = d_t[
                                :, s0 * ACC : (s0 + sz) * ACC
                            ].rearrange("p (c a) -> p c a", a=ACC)
                            w3g = wchunks[g][:, :].rearrange("p (c a) -> p c a", a=ACC)
                            nc.gpsimd.tensor_tensor(
                                out=d3g, in0=w3g, in1=pt_b, op=AluOpType.subtract
                            )
                    if CFG["gps_full"]:
                        # max|d| = max(max-tree(d), -min-tree(d)) — walrus has
                        # no abs_max TT op, so run dual max/min trees on the
                        # signed diffs, all on gpsimd: 144 -> 72 -> 36 -> 18
                        # -> 9 each, then negate + combine. Ping-pong through
                        # dead regions of d_t / t2m / t2n.
                        t2m = dwork.tile(
                            [P, COUT * (ACC // 2)], mybir.dt.float32, tag="T2", bufs=1
                        )
                        t2n = dwork.tile(
                            [P, COUT * (ACC // 2)], mybir.dt.float32, tag="T2N", bufs=1
                        )

                        def v3(tile_, off_co_elems, w):
                            return tile_[
                                :, off_co_elems : off_co_elems + COUT * w
                            ].rearrange("p (c a) -> p c a", a=w)

                        d144 = v3(d_t, 0, ACC)
                        for tm, alu in ((t2m, AluOpType.max), (t2n, AluOpType.min)):
                            nc.gpsimd.tensor_tensor(
                                out=v3(tm, 0, 72),
                                in0=d144[:, :, 0:72],
                                in1=d144[:, :, 72:144],
                                op=alu,
                            )
                        if CFG.get("gps_stop_w", 9) == 36:
                            # shallow: one more level per tree, park both
                            # 36-wide remainders; DVE does dual reduces
                            for tm, alu, off36 in (
                                (t2m, AluOpType.max, 0),
                                (t2n, AluOpType.min, COUT * 36),
                            ):
                                s = v3(tm, 0, 72)
                                nc.gpsimd.tensor_tensor(
                                    out=v3(d_t, off36, 36),
                                    in0=s[:, :, 0:36],
                                    in1=s[:, :, 36:72],
                                    op=alu,
                                )
                            r36 = outp.tile(
                                [P, COUT * 72], mybir.dt.float32, tag="r36", bufs=2
                            )
                            nc.gpsimd.tensor_copy(
                                out=r36[:, :], in_=d_t[:, 0 : COUT * 72]
                            )
                            pending.append((t, ("r36", r36)))
                            continue_gps = True
                        else:
                            continue_gps = False
                        if not continue_gps:
                            # deep: max chain t2m(72) -> d[0:](36) -> t2m(18)
                            # -> d[0:](9); min chain t2n(72) -> d[2304:](36)
                            # -> t2n(18) -> d[576:](9)
                            for tm, alu, off36, off9 in (
                                (t2m, AluOpType.max, 0, 0),
                                (t2n, AluOpType.min, COUT * 36, COUT * 9),
                            ):
                                s = v3(tm, 0, 72)
                                nc.gpsimd.tensor_tensor(
                                    out=v3(d_t, off36, 36),
                                    in0=s[:, :, 0:36],
                                    in1=s[:, :, 36:72],
                                    op=alu,
                                )
                                s = v3(d_t, off36, 36)
                                nc.gpsimd.tensor_tensor(
                                    out=v3(tm, 0, 18),
                                    in0=s[:, :, 0:18],
                                    in1=s[:, :, 18:36],
                                    op=alu,
                                )
                                s = v3(tm, 0, 18)
                                nc.gpsimd.tensor_tensor(
                                    out=v3(d_t, off9, 9),
                                    in0=s[:, :, 0:9],
                                    in1=s[:, :, 9:18],
                                    op=alu,
                                )
                            # -min9 then combine into m9 (in place on gpsimd)
                            neg_off = COUT * 18
                            nc.gpsimd.tensor_tensor(
                                out=d_t[:, neg_off : neg_off + COUT * 9],
                                in0=zeros9[:, :],
                                in1=d_t[:, COUT * 9 : COUT * 18],
                                op=AluOpType.subtract,
                            )
                            nc.gpsimd.tensor_tensor(
                                out=d_t[:, 0 : COUT * 9],
                                in0=d_t[:, 0 : COUT * 9],
                                in1=d_t[:, neg_off : neg_off + COUT * 9],
                                op=AluOpType.max,
                            )
                            # park the 9-wide result so the big ping-pong
                            # buffers free immediately and the DVE reduce can
                            # be deferred (flush_keep) without blocking
                            r9 = outp.tile(
                                [P, COUT * 9], mybir.dt.float32, tag="r9"
                            )
                            nc.gpsimd.tensor_copy(
                                out=r9[:, :], in_=d_t[:, 0 : COUT * 9]
                            )
                            pending.append(
                                (t, r9[:, :].rearrange("p (c a) -> p c a", a=9))
                            )
                    else:
                        pending.append(
                            (t, d_t[:, :].rearrange("p (c a) -> p c a", a=ACC))
                        )
                else:
                    # mix tiles donate their last weight chunk to gpsimd
                    n_sq = COUT - cog if t in mix_tiles else COUT
                    dist = outp.tile([P, COUT], mybir.dt.float32, tag="dist")
                    dm = None
                    pt_b = pt[:, :].unsqueeze(1).broadcast_to([P, cog, ACC])
                    if t in mix_tiles:
                        dm = dwork.tile([P, cog * ACC], mybir.dt.float32, tag="Dm")
                        w3g = wchunks[nch - 1][:, :].rearrange(
                            "p (c a) -> p c a", a=ACC
                        )
                        nc.gpsimd.tensor_tensor(
                            out=dm[:, :].rearrange("p (c a) -> p c a", a=ACC),
                            in0=w3g,
                            in1=pt_b,
                            op=AluOpType.subtract,
                        )
                    # segmented scan-max: one instruction per cout-chunk,
                    # whose step-0-inner output AP drops each page's final
                    # running max into dist[:, co]
                    if wfull is not None and t >= wfull_from and n_sq == COUT:
                        plan = [(0, COUT, wfull)]
                    else:
                        plan = [
                            (starts[g], chunk_sizes[g], wchunks[g])
                            for g in range(nch)
                            if starts[g] + chunk_sizes[g] <= n_sq
                        ]
                    for s0, sz, wt in plan:
                        d0 = dist[:, s0 : s0 + sz]
                        squash = bass.AP(
                            tensor=d0.tensor,
                            offset=d0.offset,
                            ap=[d0.ap[0], [1, sz], [0, ACC]],
                        )
                        w3 = wt[:, : sz * ACC].rearrange("p (c a) -> p c a", a=ACC)
                        ptb = pt[:, :].unsqueeze(1).broadcast_to([P, sz, ACC])
                        nc.vector._custom_dve(op, out=squash, in0=w3, in1=ptb)
                    sq_bias_eng = (
                        nc.gpsimd if CFG.get("sq_bias_gps", False) else nc.vector
                    )
                    sq_bias_eng.tensor_tensor(
                        out=dist[:, 0:n_sq],
                        in0=dist[:, 0:n_sq],
                        in1=bias_rep[:, 0:n_sq],
                        op=AluOpType.add,
                    )
                    if dm is not None:
                        r3 = dm[:, :].rearrange("p (c a) -> p c a", a=ACC)
                        nc.vector.tensor_reduce(
                            out=dist[:, n_sq:COUT],
                            in_=r3,
                            axis=mybir.AxisListType.X,
                            op=AluOpType.max,
                            apply_absolute_value=True,
                        )
                        nc.vector.tensor_tensor(
                            out=dist[:, n_sq:COUT],
                            in0=dist[:, n_sq:COUT],
                            in1=bias_rep[:, n_sq:COUT],
                            op=AluOpType.add,
                        )
                    nc.sync.dma_start(
                        out=out_d[t * P : (t + 1) * P, :], in_=dist[:, :]
                    )
                    flush_pending(keep=CFG.get("flush_keep", 0))
            flush_pending()
    nc.compile()
    return nc


def _host_prep(inputs):
    x = np.asarray(inputs["x"], dtype=np.float32)
    weights = np.asarray(inputs["weights"], dtype=np.float32)
    bias = np.asarray(inputs["bias"], dtype=np.float32)
    assert x.shape == (B, C, H, W)
    assert weights.shape == (COUT, ACC)

    x_pad = np.pad(x, ((0, 0), (0, 0), (1, 1), (1, 1)), mode="edge")
    from numpy.lib.stride_tricks import sliding_window_view

    pw = sliding_window_view(x_pad, (K, K), axis=(2, 3))  # (B, C, HOUT, WOUT, K, K)
    patches = np.ascontiguousarray(pw.transpose(0, 2, 3, 1, 4, 5)).reshape(
        B, NPOS, ACC
    )
    wflat = np.ascontiguousarray(weights.reshape(1, COUT * ACC))
    bflat = np.ascontiguousarray(bias.reshape(1, COUT))
    return patches, wflat, bflat


_NC_CACHE = None


def _get_nc():
    global _NC_CACHE
    if _NC_CACHE is None:
        _NC_CACHE = _build_bass()
    return _NC_CACHE


def _run(inputs, trace=False):
    from concourse.bass_utils import run_bass_kernel_spmd

    patches, wflat, bflat = _host_prep(inputs)
    in_maps = []
    for core in range(NCORES):
        b, half = core // HALVES, core % HALVES
        shard = np.ascontiguousarray(
            patches[b, half * POS_PER_CORE : (half + 1) * POS_PER_CORE, :]
        )
        in_maps.append({"patches": shard, "w": wflat, "bias": bflat})

    nc = _get_nc()
    res = run_bass_kernel_spmd(nc, in_maps, core_ids=list(range(NCORES)), trace=trace)

    rows_per_half = POS_PER_CORE // WOUT  # 32
    out = np.empty((B, COUT, HOUT, WOUT), dtype=np.float32)
    for core in range(NCORES):
        b, half = core // HALVES, core % HALVES
        o = res.results[core]["out"]  # [POS_PER_CORE, COUT]
        out[b, :, half * rows_per_half : (half + 1) * rows_per_half, :] = o.T.reshape(
            COUT, rows_per_half, WOUT
        )
    return out, res


def kernel(**inputs) -> np.ndarray:
    out, _ = _run(inputs, trace=_TRACE)
    return out

